# revision 7
# baseline (speedup 1.0000x reference)
"""Trainium2 Bass kernel: grouped-pointwise FFN with channel shuffle.

Computes (per batch b, all ops pointwise in T):
    h   = W1_grouped @ (x * mask) + b1          # G=4 block-diagonal GEMM
    h   = channel_shuffle(h, G)
    h   = gelu(h)                               # exact erf gelu
    out = (W2_grouped @ h + b2) * mask

Sharding: data-parallel over batch B=16 across 8 cores (2 batches/core).
Weights are replicated; no collectives.

Layout on device (channel-partition):
  GEMM1: lhsT = w1 block [K=128(cin/G), M=128(out-ch block)],
         rhs  = x tile [128, 512(T chunk)], PSUM out [128, 512].
  gelu+bias fused on ScalarE reading PSUM [128, 1024] spans (2 banks).
  Channel shuffle is free: GEMM2's weight blocks are pre-gathered on the
  host so that GEMM2 group g2 contracts directly over GEMM1's (g, m=g2)
  output tiles.
  GEMM2: accumulate 4 K-blocks into PSUM [128, 512]; drain with a single
  fused DVE op: out = (psum + b2) * mask.

Matmuls stream float32r (fp32 at 1 cycle/row vs 4 for float32; measured
end-to-end rel err ~2e-4 vs fp32 reference). All tensors feeding matmuls
are typed float32r end-to-end (BIR verifier requirement).

DMA: inputs/weights on the SP HWDGE ring (small tensors first, weights
chunked in use-order), outputs on the otherwise-idle GpSimd SWDGE ring.
A burst of tiny warm-up matmuls keeps the PE HAM clock-gate warm before
the first real GEMM.
"""

import numpy as np

import concourse.mybir as mybir
import concourse.tile as tile
from concourse import bacc
from concourse import bass_utils

F32 = mybir.dt.float32
F32R = mybir.dt.float32r

N_CORES = 8
B, CIN, T = 16, 512, 2048
H, COUT, G = 2048, 512, 4
BPC = B // N_CORES        # batches per core
CH = 512                  # T chunk (= max fp32 matmul free dim = 1 PSUM bank)
NCH = T // CH             # 4 chunks
MB = (H // G) // 128      # 4 output-channel blocks per group in GEMM1
GELU_W = 1024             # ACT op width (2 PSUM banks)
XCH = 1024                # x / out DMA chunk width
N_WARMUP = 12             # tiny matmuls to warm the PE clock gate

MM_DT = F32R

_compiled = {}


def _build(mm_dt):
    nc = bacc.Bacc(
        "TRN2", target_bir_lowering=False, debug=False, num_devices=N_CORES
    )
    xs = nc.dram_tensor("xs", [BPC * G, 128, T], mm_dt, kind="ExternalInput").ap()
    mk = nc.dram_tensor("mk", [BPC, T], F32, kind="ExternalInput").ap()
    mkr = nc.dram_tensor("mkr", [BPC, T], mm_dt, kind="ExternalInput").ap()
    ones = nc.dram_tensor("ones", [1, 128], mm_dt, kind="ExternalInput").ap()
    # w1t columns are (m, g, o)-major so the m=0 block is one contiguous
    # 512-col DMA needed first; w2t columns are (g2, g, o)-major.
    w1t = nc.dram_tensor("w1t", [128, G * MB * 128], mm_dt, kind="ExternalInput").ap()
    w2t = nc.dram_tensor("w2t", [128, G * G * 128], mm_dt, kind="ExternalInput").ap()
    b1t = nc.dram_tensor("b1t", [128, G * MB], F32, kind="ExternalInput").ap()
    b2t = nc.dram_tensor("b2t", [128, G], F32, kind="ExternalInput").ap()
    outs = nc.dram_tensor("outs", [BPC * G, 128, T], F32, kind="ExternalOutput").ap()

    with tile.TileContext(nc) as tc:
        with (
            tc.tile_pool(name="consts", bufs=1) as cpool,
            tc.tile_pool(name="xp", bufs=BPC * G) as xpool,
            tc.tile_pool(name="mbcp", bufs=2) as mbpool,
            tc.tile_pool(name="mkrp", bufs=2) as mkrpool,
            tc.tile_pool(name="hp", bufs=2 * G) as hpool,
            tc.tile_pool(name="op", bufs=2) as opool,
            tc.tile_pool(name="ps1p", bufs=3, space="PSUM") as ps1pool,
            tc.tile_pool(name="ps2p", bufs=2, space="PSUM") as ps2pool,
        ):
            # small constants first so they clear the DMA ring quickly
            ones_sb = cpool.tile([1, 128], mm_dt)
            nc.sync.dma_start(ones_sb, ones)
            b1_sb = cpool.tile([128, G * MB], F32)
            nc.sync.dma_start(b1_sb, b1t)
            b2_sb = cpool.tile([128, G], F32)
            nc.sync.dma_start(b2_sb, b2t)

            # PE warm-up: tiny matmuls on the ones row keep the HAM
            # activity window busy while real inputs stream in.
            wps = ps2pool.tile([128, 128], F32, tag="ps2", name="wps")
            for i in range(N_WARMUP):
                nc.tensor.matmul(
                    wps[:, 0:128], ones_sb, ones_sb, start=True, stop=True
                )

            w1_sb = cpool.tile([128, G * MB * 128], mm_dt)
            w2_sb = cpool.tile([128, G * G * 128], mm_dt)

            x_sb = [[None] * G for _ in range(BPC)]
            mask_bc = [None] * BPC

            def prep_batch(b):
                # mask row -> broadcast across 128 partitions via K=1
                # f32r matmuls, chunk by chunk; x loads and mask muls are
                # chunked so the first GEMM1 matmul can start early.
                mkrow = mkrpool.tile([1, T], mm_dt, tag="mkr", name="mkrow")
                nc.sync.dma_start(mkrow, mkr[b : b + 1, :])
                mbc = mbpool.tile([128, T], F32, tag="mbc", name="mbc")
                nc.sync.dma_start(mbc[0:1, :], mk[b : b + 1, :])
                for c in range(NCH):
                    cs = slice(c * CH, (c + 1) * CH)
                    psb = ps2pool.tile([128, CH], F32, tag="ps2", name="psb")
                    nc.tensor.matmul(
                        psb, ones_sb, mkrow[:, cs], start=True, stop=True
                    )
                    nc.vector.tensor_copy(mbc[:, cs], psb)
                mask_bc[b] = mbc

            def load_x(b, g):
                xt = xpool.tile([128, T], mm_dt, tag="x", name="xt")
                for c in range(T // XCH):
                    cs = slice(c * XCH, (c + 1) * XCH)
                    nc.sync.dma_start(xt[:, cs], xs[b * G + g][:, cs])
                    nc.vector.tensor_mul(
                        xt[:, cs], xt[:, cs], mask_bc[b][:, cs]
                    )
                x_sb[b][g] = xt

            def load_w1(m):
                ws = slice(m * G * 128, (m + 1) * G * 128)
                nc.sync.dma_start(w1_sb[:, ws], w1t[:, ws])

            def load_w2(g2):
                ws = slice(g2 * G * 128, (g2 + 1) * G * 128)
                nc.sync.dma_start(w2_sb[:, ws], w2t[:, ws])

            def gemm1_g(b, m, g):
                # one h tile (g) for (b, m), gelu+bias fused on drain
                ht = hpool.tile([128, T], mm_dt, tag="h", name="ht")
                w_ap = w1_sb[:, (m * G + g) * 128 : (m * G + g + 1) * 128]
                for half in range(T // GELU_W):
                    ps1 = ps1pool.tile([128, GELU_W], F32, tag="ps1", name="ps1")
                    for cc in range(GELU_W // CH):
                        c = half * (GELU_W // CH) + cc
                        nc.tensor.matmul(
                            ps1[:, cc * CH : (cc + 1) * CH],
                            w_ap,
                            x_sb[b][g][:, c * CH : (c + 1) * CH],
                            start=True, stop=True,
                        )
                    nc.scalar.activation(
                        ht[:, half * GELU_W : (half + 1) * GELU_W],
                        ps1,
                        mybir.ActivationFunctionType.Gelu,
                        bias=b1_sb[:, m * G + g : m * G + g + 1],
                        scale=1.0,
                    )
                return ht

            def gemm2_chunk(b, g2, hts, ot, c):
                cs = slice(c * CH, (c + 1) * CH)
                ps2 = ps2pool.tile([128, CH], F32, tag="ps2", name="ps2")
                for g in range(G):
                    nc.tensor.matmul(
                        ps2,
                        w2_sb[:, (g2 * G + g) * 128 : (g2 * G + g + 1) * 128],
                        hts[g][:, cs],
                        start=(g == 0), stop=(g == G - 1),
                    )
                # out = (psum + b2) * mask, single fused DVE op
                nc.vector.scalar_tensor_tensor(
                    ot[:, cs],
                    ps2,
                    b2_sb[:, g2 : g2 + 1],
                    mask_bc[b][:, cs],
                    op0=mybir.AluOpType.add,
                    op1=mybir.AluOpType.mult,
                )
                if c % (XCH // CH) == (XCH // CH) - 1:
                    os_ = slice((c + 1) * CH - XCH, (c + 1) * CH)
                    nc.gpsimd.dma_start(outs[b * G + g2][:, os_], ot[:, os_])

            # head: mask prep + first x tiles + first weight block
            prep_batch(0)
            load_w1(0)
            load_x(0, 0)
            load_x(0, 1)
            load_w1(1)
            load_x(0, 2)
            load_x(0, 3)
            load_w1(2)
            load_w1(3)
            for g2 in range(G):
                load_w2(g2)

            # software pipeline over (b, m): GEMM2 chunks of iteration i-1
            # are interleaved between GEMM1 groups of iteration i so PE
            # alternates with ScalarE instead of stalling on gelu.
            prev = None
            for b in range(BPC):
                for m in range(MB):
                    hts = []
                    if prev is not None:
                        pot = opool.tile([128, T], F32, tag="o", name="pot")
                    for g in range(G):
                        hts.append(gemm1_g(b, m, g))
                        if prev is not None:
                            gemm2_chunk(prev[0], prev[1], prev[2], pot, g)
                    prev = (b, m, hts)
                    if b + 1 < BPC and m == 1:
                        # emit next batch's loads mid-stream so its DMAs
                        # and mask prep overlap this batch's compute
                        prep_batch(b + 1)
                        for g in range(G):
                            load_x(b + 1, g)
            pot = opool.tile([128, T], F32, tag="o", name="pot")
            for c in range(NCH):
                gemm2_chunk(prev[0], prev[1], prev[2], pot, c)

    nc.compile()
    return nc


def get_nc(mm_dt=None):
    mm_dt = MM_DT if mm_dt is None else mm_dt
    if mm_dt not in _compiled:
        _compiled[mm_dt] = _build(mm_dt)
    return _compiled[mm_dt]


def prep_inputs(x, x_mask, w1, b1, w2, b2):
    """Host-side layout prep. Returns per-core in_maps."""
    x = np.ascontiguousarray(np.asarray(x, dtype=np.float32))
    x_mask = np.asarray(x_mask, dtype=np.float32)
    w1 = np.asarray(w1, dtype=np.float32)
    b1 = np.asarray(b1, dtype=np.float32)
    w2 = np.asarray(w2, dtype=np.float32)
    b2 = np.asarray(b2, dtype=np.float32)

    # w1 [H, CIN/G] -> lhsT blocks [i, (m, g, o)]
    w1r = w1.reshape(G, MB, 128, CIN // G)          # g, m, o, i
    w1t = np.ascontiguousarray(
        np.transpose(w1r, (3, 1, 0, 2)).reshape(128, G * MB * 128)
    )
    # w2 [COUT, H/G] -> lhsT blocks [i_local, (g2, g, o)]
    # GEMM2 group g2 contracts h tile (g, m=g2) row r against
    # w2[g2*128+o, r*4+g] (channel shuffle pre-applied).
    w2r = w2.reshape(G, 128, 128, G)                # g2, o, r, g
    w2t = np.ascontiguousarray(
        np.transpose(w2r, (2, 0, 3, 1)).reshape(128, G * G * 128)
    )
    b1tt = np.ascontiguousarray(
        b1.reshape(G, MB, 128).transpose(2, 1, 0).reshape(128, G * MB)
    )
    b2tt = np.ascontiguousarray(b2.reshape(G, 128).T)
    ones = np.ones((1, 128), np.float32)

    xr = x.reshape(N_CORES, BPC * G, 128, T)
    mr = x_mask.reshape(N_CORES, BPC, T)

    in_maps = []
    for k in range(N_CORES):
        mk_k = np.ascontiguousarray(mr[k])
        in_maps.append(
            {
                "xs": np.ascontiguousarray(xr[k]),
                "mk": mk_k,
                "mkr": mk_k,
                "ones": ones,
                "w1t": w1t,
                "w2t": w2t,
                "b1t": b1tt,
                "b2t": b2tt,
            }
        )
    return in_maps


def assemble_output(results):
    """results: list of 8 dicts with 'outs' [BPC*G, 128, T]."""
    parts = [r["outs"].reshape(BPC, G * 128, T) for r in results]
    return np.concatenate(parts, axis=0).astype(np.float32)


def kernel(x, x_mask, w1, b1, w2, b2, n_groups):
    assert int(n_groups) == G
    import os

    # NTFF tracing needs antenv.axon_hooks, absent on this image; make
    # sure an inherited BASS_TRACE can't push us onto that path.
    os.environ["BASS_NEVER_TRACE"] = "1"
    nc = get_nc()
    in_maps = prep_inputs(x, x_mask, w1, b1, w2, b2)
    res = bass_utils.run_bass_kernel_spmd(
        nc, in_maps, core_ids=list(range(N_CORES))
    )
    return assemble_output(res.results)
